# revision 58
# baseline (speedup 1.0000x reference)
"""Trainium2 Bass kernel: two-phase LSTM encoder.

Computes, for batch B=4096, hidden H=1024:
  scan1: 8 steps of (Linear(2->256) + LSTMCell) over obs_traj_rel, carry (h0, c0)
  c_out = h1.T.reshape(B, H)
  scan2: 12 steps over pre_traj_rel, carry (h1, c1)
  x_out = h2.T.reshape(B, H)

Strategy (data-parallel over batch, 8 NeuronCores, BL=512 rows each):
  - The 256-wide input embedding is folded into the gate weights on the host:
      gates = x @ (W_ih @ W_in).T + h @ W_hh.T + (W_ih @ b_in + b)
    so the per-step device work is one K=2 matmul + one K=1024 matmul.
  - Hidden state lives in SBUF transposed ([H, BL]); gates come out as
    [4H, BL] tiles. Matmul weights are stored pre-scaled by 512 in every
    representation, and every gate activation applies scale=1/512, so fp8
    and bf16 paths can mix freely in one PSUM accumulation.
  - Precision schedule (budget: rel err < 2e-2): scan-1 amplifies injected
    errors ~1.2x/step, so its step 0 runs all-bf16; every other step (1-7 of
    scan-1, all of scan-2) runs the f/i/o gate tiles in fp8e4 DoubleRow
    (two 128-row contraction blocks per matmul -> ~2x PE rate) and keeps
    the g gate in bf16: the tanh-path is ~10x more sensitive to
    quantization noise than the sigmoid gates (measured: ifo-fp8 adds
    ~5e-3 to x_out, g-fp8 alone adds ~3.8e-2; HW errors match the numpy
    simulation to 3 digits). h is kept both as bf16 tiles (g matmuls) and
    as an fp8 k-pair tile (ifo matmuls); weights W*512 fit e4m3 (max
    |W*512| ~ 131 < 240) so no clipping loss.
  - The four K=2 input-projection matmuls of a j-group are row-packed into
    disjoint 32-row strips of the PE array (tile_position=(32r,0)), so they
    run concurrently (~1 matmul's streaming time instead of 4).
  - Weight columns are permuted on the host so each j-group's gate blocks
    are contiguous; weights DMA in (k, j) chunks so compute starts after
    ~1MB instead of the full set.
  - h is written back as bf16 (next step's matmul operand); the scan-final h
    is additionally produced in fp32 and DMA'd out as [H, BL]; the host
    concatenation of those per-core blocks is exactly h.T, so c_out/x_out are
    free reshapes.
"""

import numpy as np
import ml_dtypes

T_OBS, T_PRE, B = 8, 12, 4096
FEAT, H = 2, 1024
N_CORES = 8
BL = B // N_CORES        # 512 batch rows per core
KB = H // 128            # 8 contraction blocks over H
NG = 4 * H // 128        # 32 gate row-tiles over 4H
WS = 512.0               # weight pre-scale; activations use scale=1/WS

# Gate-block permutation: new column block p = 4*j + gi holds original gate
# block (f,i,g,o)[gi] for H-rows j*128..(j+1)*128, i.e. original m-index
# _PERM[p]. Gates appear in (f,i,g,o) order so the forget gate (first in the
# cell-update chain) finishes earliest.
_GATE_ORIG = (1, 0, 2, 3)  # f,i,g,o -> original gate index (i,f,g,o order)
_PERM = [g0 * KB + j for j in range(KB) for g0 in _GATE_ORIG]
# fp8 weight tensor for scan 2: only the f,i,o blocks, 3 per j-group
_IFO_ORIG = (1, 0, 3)
_PERM8 = [g0 * KB + j for j in range(KB) for g0 in _IFO_ORIG]
# bf16 g-gate weight tensor for scan 2
_PERMG = [2 * KB + j for j in range(KB)]

_BF16 = ml_dtypes.bfloat16
_F8E4 = ml_dtypes.float8_e4m3
_CACHE = {}
# DoubleRowSwInterleave: weights pre-interleaved on the host
# (A127,B127,A126,B126,... per k-pair) so the weight load is a contiguous
# read instead of the hardware interleave pattern.
_SWI = False


def _swi_pack(w8):
    """[H, 24*128] fp8 block layout -> [4, 128, 24*256] sw-interleaved."""
    w = np.asarray(w8)
    # w[128*kb + p, q*128 + m]
    w4 = w.reshape(4, 2, 128, 24, 128)          # [b, i, p, q, m]
    rev = w4[:, :, :, :, ::-1]                  # m -> 127-u
    # target [b, p, q, u, i]
    out = rev.transpose(0, 2, 3, 4, 1)
    return np.ascontiguousarray(out.reshape(4, 128, 24 * 256))


def _build_nc():
    import concourse.tile as tile
    from concourse import bacc, mybir

    f32 = mybir.dt.float32
    bf16 = mybir.dt.bfloat16
    f8e4 = mybir.dt.float8e4
    SIG = mybir.ActivationFunctionType.Sigmoid
    TANH = mybir.ActivationFunctionType.Tanh
    DR = mybir.MatmulPerfMode.DoubleRow

    nc = bacc.Bacc(
        "TRN2", target_bir_lowering=False, debug=False, enable_asserts=False
    )

    DR_MODE = (
        mybir.MatmulPerfMode.DoubleRowSwInterleave if _SWI else DR
    )
    w8_dram_shape = [4, 128, 24 * 256] if _SWI else [H, 24 * 128]
    d_g0 = nc.dram_tensor("g0_obs", [4 * H, BL], bf16, kind="ExternalInput").ap()
    d_wgo = nc.dram_tensor("wg_obs", [H, 8 * 128], bf16, kind="ExternalInput").ap()
    d_w8o = nc.dram_tensor("w8_obs", w8_dram_shape, f8e4, kind="ExternalInput").ap()
    d_w8 = nc.dram_tensor("w8_pre", w8_dram_shape, f8e4, kind="ExternalInput").ap()
    d_wg = nc.dram_tensor("wg_pre", [H, 8 * 128], bf16, kind="ExternalInput").ap()
    d_wx = [
        nc.dram_tensor(f"wx_{s}", [128, 4 * H], bf16, kind="ExternalInput").ap()
        for s in ("obs", "pre")
    ]
    d_bias = [
        nc.dram_tensor(f"bias_{s}", [128, NG], f32, kind="ExternalInput").ap()
        for s in ("obs", "pre")
    ]
    d_x = [
        nc.dram_tensor(f"x_{s}", [t, FEAT, BL], bf16, kind="ExternalInput").ap()
        for s, t in zip(("obs", "pre"), (T_OBS, T_PRE))
    ]
    d_c0 = nc.dram_tensor("c0T", [H, BL], f32, kind="ExternalInput").ap()
    d_c1 = nc.dram_tensor("c1T", [H, BL], f32, kind="ExternalInput").ap()
    d_hout = [
        nc.dram_tensor(f"h{i}T", [H, BL], f32, kind="ExternalOutput").ap()
        for i in (1, 2)
    ]

    with tile.TileContext(nc) as tc:
        with (
            tc.tile_pool(name="wp", bufs=1) as wp,
            tc.tile_pool(name="hp", bufs=18) as hp,
            tc.tile_pool(name="h8p", bufs=2) as h8p,
            tc.tile_pool(name="cp", bufs=1) as cp,
            tc.tile_pool(name="gp", bufs=12) as gp,
            tc.tile_pool(name="g0p", bufs=4) as g0p,
            tc.tile_pool(name="xp", bufs=4) as xp,
            tc.tile_pool(name="pp", bufs=2, space="PSUM") as pp,
        ):
            # Persistent weights. Scan-1 set loads in (j-chunk)-major order so
            # the first j-group can start after ~1MB; scan-2 set is emitted
            # inside the scan loop (after step 1) so its DMA traffic queues
            # behind the critical first-step loads.
            wgo_sb = wp.tile([128, KB, 8 * 128], bf16, tag="wgo", name="wgo")
            w8_shape = [128, 4, 24, 2, 128] if _SWI else [128, KB, 24 * 128]
            w8o_sb = wp.tile(w8_shape, f8e4, tag="w8o", name="w8o")
            w8_sb = wp.tile(w8_shape, f8e4, tag="w8", name="w8")

            def load_w8(sb, dram):
                if _SWI:
                    for b in range(4):
                        eng = nc.sync if b % 2 == 0 else nc.vector
                        eng.dma_start(out=sb[:, b, :, :, :], in_=dram[b, :, :])
                else:
                    for k in range(KB):
                        nc.sync.dma_start(
                            out=sb[:, k, :], in_=dram[k * 128 : (k + 1) * 128, :]
                        )

            def dr_weight_ap(sb, b, q):
                if _SWI:
                    return sb[:, b, q, :, :]
                return sb[:, 2 * b : 2 * b + 2, q * 128 : (q + 1) * 128]
            wg_sb = wp.tile([128, KB, 8 * 128], bf16, tag="wg", name="wg")
            wx_sb = [
                wp.tile([128, 4 * H], bf16, tag=f"wx{s}", name=f"wx{s}")
                for s in range(2)
            ]
            bias_sb = [
                wp.tile([128, NG], f32, tag=f"bias{s}", name=f"bias{s}")
                for s in range(2)
            ]

            def load_weight_set0():
                # Step 0's gate pre-activations are host-computed (d_g0), so
                # scan-1 only needs wx, the fp8 ifo weights and the bf16 g
                # weights. wx rides the near-empty gpsimd queue; the j-major
                # chunking of w8o/wgo matches step-1's consumption order.
                nc.sync.dma_start(out=bias_sb[0], in_=d_bias[0][:, :])
                for j in range(KB):
                    jsl = slice(j * 512, (j + 1) * 512)
                    nc.gpsimd.dma_start(out=wx_sb[0][:, jsl], in_=d_wx[0][:, jsl])
                # interleave the ifo-fp8 and g-bf16 chunks j-major so step 1
                # unblocks j-group by j-group while step 0's ACTs run
                for j in range(KB):
                    qsl = slice(3 * j * 128, 3 * (j + 1) * 128)
                    gsl = slice(j * 128, (j + 1) * 128)
                    for k in range(KB):
                        nc.scalar.dma_start(
                            out=wgo_sb[:, k, gsl],
                            in_=d_wgo[k * 128 : (k + 1) * 128, gsl],
                        )
                    for k in range(KB):
                        nc.scalar.dma_start(
                            out=w8o_sb[:, k, qsl],
                            in_=d_w8o[k * 128 : (k + 1) * 128, qsl],
                        )

            def load_weight_set1():
                nc.sync.dma_start(out=wx_sb[1], in_=d_wx[1][:, :])
                nc.sync.dma_start(out=bias_sb[1], in_=d_bias[1][:, :])
                load_w8(w8_sb, d_w8)
                for k in range(KB):
                    nc.sync.dma_start(
                        out=wg_sb[:, k, :], in_=d_wg[k * 128 : (k + 1) * 128, :]
                    )

            # PE warm-up: the array sits idle ~10us waiting for the first
            # weight/h0 DMAs, and would then start the real stream at the
            # HAM-throttled 1.2 GHz clock. Dummy matmuls on zeroed tiles
            # (never read) during that window flip the clock gate to 8/8
            # before the real stream begins.
            wu_w = xp.tile([128, 128], bf16, tag="x", name="wu_w")
            wu_w8 = xp.tile([128, 128], f8e4, tag="x8", name="wu_w8")
            wu_rhs = xp.tile([128, BL], bf16, tag="x", name="wu_rhs")
            nc.vector.memset(wu_w, 0.0)
            nc.vector.memset(wu_w8, 0.0)
            nc.vector.memset(wu_rhs, 0.0)
            wu_p = pp.tile([128, 4 * BL], f32, tag="ps", name="wu_p")
            for _ in range(28):
                nc.tensor.matmul(
                    wu_p[:, :BL], wu_w, wu_rhs, start=True, stop=True
                )

            # c0 first so it leads the scalar DMA queue (w8o/wgo share it
            # and must not starve it)
            h_cur = None
            c_sb = [
                cp.tile([128, BL], f32, tag=f"c{j}", name=f"c{j}") for j in range(KB)
            ]
            for j in range(KB):
                nc.scalar.dma_start(out=c_sb[j], in_=d_c0[j * 128 : (j + 1) * 128, :])

            load_weight_set0()

            h8_cur = None
            steps = []
            for s, T, d_xs, d_out in (
                (0, T_OBS, d_x[0], d_hout[0]),
                (1, T_PRE, d_x[1], d_hout[1]),
            ):
                for t in range(T):
                    steps.append((s, t, T, d_xs, d_out))

            def make_xt(si):
                s, t, T, d_xs, d_out = steps[si]
                xt = xp.tile([128, BL], bf16, tag="x", name=f"x_{s}_{t}")
                for r in range(4):
                    nc.gpsimd.dma_start(
                        out=xt[32 * r : 32 * r + FEAT, :], in_=d_xs[t, :, :]
                    )
                return xt

            def emit_x(si, xt, j):
                # One 4-bank PSUM tile per j-group: the four gate psums
                # allocate atomically, so the four row-packed K=2
                # input-projection matmuls become ready together and stream
                # concurrently in disjoint 32-row strips. Emitted one
                # j-group EARLY (pipelined), so the 4 weight-strip loads
                # ride the previous group's cheap-LDW g phase instead of
                # stalling this group's entry.
                s, t, T, d_xs, d_out = steps[si]
                big = pp.tile(
                    [128, 4 * BL], f32, tag="ps", name=f"ps_{s}_{t}_{j}"
                )
                ps = []
                for gi in range(4):
                    p = big[:, gi * BL : (gi + 1) * BL]
                    m = 4 * j + gi
                    msl = slice(m * 128, (m + 1) * 128)
                    rsl = slice(32 * gi, 32 * gi + FEAT)
                    nc.tensor.matmul(
                        p,
                        wx_sb[s][rsl, msl],
                        xt[rsl, :],
                        start=True,
                        stop=False,
                        tile_position=(32 * gi, 0),
                        skip_group_check=True,
                    )
                    ps.append(p)
                return ps

            # step 0 consumes no x (host-computed gates); start the x
            # pipeline at step 1
            xt_cur = make_xt(1)
            xt_next = None
            pending_ps = None

            for si, (s, t, T, d_xs, d_out) in enumerate(steps):
                if True:
                    if s == 1 and t == 0:
                        # fresh cell state for scan 2 (WAR on scan-1 reads)
                        for j in range(KB):
                            nc.scalar.dma_start(
                                out=c_sb[j], in_=d_c1[j * 128 : (j + 1) * 128, :]
                            )
                    if s == 0 and t == 1:
                        load_weight_set1()

                    last = t == T - 1
                    # step modes: scan-1 step 0 all-bf16 (early errors are
                    # amplified ~1.2x/step through the scan), everything else
                    # f/i/o-fp8 + g-bf16
                    fp8_step = not (s == 0 and t < 1)
                    produce_next = not (s == 1 and last)
                    produce_h8 = produce_next
                    h_next = (
                        [
                            hp.tile([128, BL], bf16, tag="h", name=f"h_{s}_{t}_{k}")
                            for k in range(KB)
                        ]
                        if produce_next
                        else None
                    )
                    h8_next = (
                        h8p.tile([128, KB, BL], f8e4, tag="h8", name=f"h8_{s}_{t}")
                        if produce_h8
                        else None
                    )

                    def emit_x_ahead(j):
                        # emit the x pack of the group after (si, j), if any
                        nonlocal pending_ps, xt_next
                        if j + 1 < KB:
                            pending_ps = emit_x(si, xt_cur, j + 1)
                        elif si + 1 < len(steps):
                            xt_next = make_xt(si + 1)
                            pending_ps = emit_x(si + 1, xt_next, 0)
                        else:
                            pending_ps = None

                    def emit_epilogue(j, ps, act_order, use_bias=True):
                        # permuted layout: gate gi of j-group j is column
                        # block m = 4*j + gi, gi in (f,i,g,o) order
                        ms = [4 * j + gi for gi in range(4)]
                        bs = bias_sb[s]

                        def _act(p_in, m, func, nm):
                            o = gp.tile([128, BL], f32, tag="g", name=nm)
                            nc.scalar.activation(
                                out=o,
                                in_=p_in,
                                func=func,
                                bias=bs[:, m : m + 1] if use_bias else 0.0,
                                scale=1.0 / WS,
                            )
                            return o

                        # emit gate activations in psum-close order (the
                        # scalar queue is strict FIFO)
                        acts = {}
                        for gi in act_order:
                            func = TANH if gi == 2 else SIG
                            acts[gi] = _act(
                                ps[gi], ms[gi], func, f"a{gi}_{s}_{t}_{j}"
                            )
                        sf, si, tg, so = (acts[gi] for gi in range(4))

                        t1 = gp.tile([128, BL], f32, tag="g", name=f"t1_{s}_{t}_{j}")
                        nc.vector.tensor_mul(t1, sf, c_sb[j])
                        t2 = gp.tile([128, BL], f32, tag="g", name=f"t2_{s}_{t}_{j}")
                        nc.vector.tensor_mul(t2, si, tg)
                        nc.vector.tensor_add(c_sb[j], t1, t2)
                        tc_j = gp.tile([128, BL], f32, tag="g", name=f"tc_{s}_{t}_{j}")
                        nc.scalar.activation(out=tc_j, in_=c_sb[j], func=TANH)

                        # next-step h first: it is on the critical path; the
                        # fp32 output copy and its DMA are not.
                        if produce_next:
                            nc.vector.tensor_mul(h_next[j], so, tc_j)
                        if produce_h8:
                            nc.vector.tensor_mul(h8_next[:, j, :], so, tc_j)
                        if last:
                            hf = gp.tile([128, BL], f32, tag="g", name=f"hf_{s}_{j}")
                            nc.vector.tensor_mul(hf, so, tc_j)
                            nc.sync.dma_start(
                                out=d_out[j * 128 : (j + 1) * 128, :], in_=hf
                            )

                    if not fp8_step:
                        # step 0: gate pre-activations are host-computed
                        # (g0 = Weff@x0 + W@h0 + b, scaled by WS, bias
                        # folded). ACT reads them straight from SBUF; no
                        # matmuls. A per-j dummy matmul on the freshly
                        # written h8 keeps the PE's HAM activity window busy
                        # so step 1 starts at the full 2.4 GHz clock.
                        for j in range(KB):
                            g0 = g0p.tile(
                                [128, 4 * BL], bf16, tag="g0", name=f"g0_{j}"
                            )
                            for gi in range(4):
                                nc.sync.dma_start(
                                    out=g0[:, gi * BL : (gi + 1) * BL],
                                    in_=d_g0[
                                        (4 * j + gi) * 128 : (4 * j + gi + 1)
                                        * 128,
                                        :,
                                    ],
                                )
                            ps = [
                                g0[:, gi * BL : (gi + 1) * BL] for gi in range(4)
                            ]
                            emit_epilogue(j, ps, (0, 1, 2, 3), use_bias=False)
                            nc.tensor.matmul(
                                wu_p[:, :BL],
                                wu_w8,
                                h8_next[:, j, :],
                                start=True,
                                stop=True,
                            )
                    else:
                        w8s = w8o_sb if s == 0 else w8_sb

                        def emit_g(ps, j, k):
                            wgs = wgo_sb if s == 0 else wg_sb
                            wg_ap = wgs[:, k, j * 128 : (j + 1) * 128]
                            nc.tensor.matmul(
                                ps[2],
                                wg_ap,
                                h_cur[k],
                                start=False,
                                stop=(k == KB - 1),
                                skip_group_check=True,
                            )

                        # Per j-group: x pack, then the g chain (cheap FWL
                        # weight loads let the LDW port prefetch the 256-col
                        # DoubleRow weights during the g stream), then the
                        # f/i/o DoubleRow chains round-robined; the NEXT
                        # group's x pack emits between g and DR. (Tested
                        # worse: interleaving g into the DR phase — each
                        # Normal<->DoubleRow transition costs ~100ns — and
                        # pair-wise j-group phases, which serialize on PSUM.)
                        for j in range(KB):
                            ps = pending_ps or emit_x(si, xt_cur, j)
                            for k in range(KB):
                                emit_g(ps, j, k)
                            emit_x_ahead(j)
                            for b in range(KB // 2):
                                for gidx, gi in enumerate((0, 1, 3)):
                                    nc.tensor.matmul(
                                        ps[gi],
                                        dr_weight_ap(w8s, b, 3 * j + gidx),
                                        h8_cur[:, 2 * b : 2 * b + 2, :],
                                        start=False,
                                        stop=(b == KB // 2 - 1),
                                        perf_mode=DR_MODE,
                                        skip_group_check=True,
                                    )
                            emit_epilogue(j, ps, (2, 0, 1, 3))

                    if xt_next is not None:
                        xt_cur = xt_next
                        xt_next = None
                    if produce_next:
                        h_cur = h_next
                    if produce_h8:
                        h8_cur = h8_next

    nc.compile()
    return nc


def _prep_host(inputs):
    inputs = {k: np.asarray(v) for k, v in inputs.items()}
    f32 = np.float32
    W_in = inputs["W_in"].astype(np.float64)
    b_in = inputs["b_in"].astype(np.float64)

    shared = {}
    for tag in ("obs", "pre"):
        W_ih = inputs[f"W_ih_{tag}"].astype(np.float64)
        W_hh = inputs[f"W_hh_{tag}"].astype(f32)
        b = inputs[f"b_{tag}"].astype(np.float64)
        W_eff = (W_ih @ W_in).astype(f32)        # [4H, FEAT]
        b_eff = (W_ih @ b_in + b).astype(f32)    # [4H]
        # weight representations pre-scaled by WS; activations divide back
        wT = np.ascontiguousarray(W_hh.T) * f32(WS)          # [H, 4H]
        wT_blocks = wT.reshape(H, NG, 128)
        w8 = np.clip(
            np.ascontiguousarray(wT_blocks[:, _PERM8, :].reshape(H, 24 * 128)),
            -240.0,
            240.0,
        ).astype(_F8E4)
        shared[f"w8_{tag}"] = _swi_pack(w8) if _SWI else w8
        wg = wT_blocks[:, _PERMG, :].reshape(H, 8 * 128)
        shared[f"wg_{tag}"] = np.ascontiguousarray(wg).astype(_BF16)
        wx_p = np.zeros((128, 4 * H), f32)
        wx_src = (W_eff.T * WS).reshape(FEAT, NG, 128)[:, _PERM, :].reshape(
            FEAT, 4 * H
        )
        for r in range(4):
            wx_p[32 * r : 32 * r + FEAT] = wx_src
        bias_p = b_eff.reshape(NG, 128)[_PERM, :].T  # [128, NG]
        shared[f"wx_{tag}"] = np.ascontiguousarray(wx_p).astype(_BF16)
        shared[f"bias_{tag}"] = np.ascontiguousarray(bias_p)

    obs = inputs["obs_traj_rel"].astype(f32)
    pre = inputs["pre_traj_rel"].astype(f32)
    h0 = inputs["h0"].astype(f32)
    c0 = inputs["c0"].astype(f32)
    c1 = inputs["c1"].astype(f32)

    # step 0 of scan-1 is a pure affine map of the inputs: compute its gate
    # pre-activations exactly on the host (this removes the 8MB bf16 weight
    # load and all step-0 matmuls from the device)
    W_ih_o = inputs["W_ih_obs"].astype(f32)
    xe0 = obs[0] @ inputs["W_in"].astype(f32).T + inputs["b_in"].astype(f32)
    g0 = (
        xe0 @ W_ih_o.T
        + h0 @ inputs["W_hh_obs"].astype(f32).T
        + inputs["b_obs"].astype(f32)
    )  # [B, 4H], original (i,f,g,o) block order
    g0_blocks = (g0.T * f32(WS)).reshape(NG, 128, B)[_PERM, :, :]  # [NG,128,B]

    in_maps = []
    for c in range(N_CORES):
        sl = slice(c * BL, (c + 1) * BL)
        m = dict(shared)
        m["x_obs"] = np.ascontiguousarray(obs[:, sl, :].transpose(0, 2, 1)).astype(
            _BF16
        )
        m["x_pre"] = np.ascontiguousarray(pre[:, sl, :].transpose(0, 2, 1)).astype(
            _BF16
        )
        m["g0_obs"] = np.ascontiguousarray(
            g0_blocks[:, :, sl].reshape(4 * H, BL)
        ).astype(_BF16)
        m["c0T"] = np.ascontiguousarray(c0[sl].T)
        m["c1T"] = np.ascontiguousarray(c1[sl].T)
        in_maps.append(m)
    return in_maps


def _run(inputs, trace=False):
    from concourse import bass_utils

    nc = _CACHE.get("nc")
    if nc is None:
        nc = _build_nc()
        _CACHE["nc"] = nc
    in_maps = _prep_host(inputs)
    res = bass_utils.run_bass_kernel_spmd(
        nc, in_maps, core_ids=list(range(N_CORES)), trace=trace
    )
    h1 = np.concatenate([r["h1T"] for r in res.results], axis=1)  # [H, B] == h1.T
    h2 = np.concatenate([r["h2T"] for r in res.results], axis=1)
    c_out = np.ascontiguousarray(h1.reshape(B, H), dtype=np.float32)
    x_out = np.ascontiguousarray(h2.reshape(B, H), dtype=np.float32)
    return (c_out, x_out), res


def kernel(**inputs):
    out, _ = _run(inputs, trace=False)
    return out


# revision 59
# speedup vs baseline: 1.0850x; 1.0850x over previous
"""Trainium2 Bass kernel: two-phase LSTM encoder.

Computes, for batch B=4096, hidden H=1024:
  scan1: 8 steps of (Linear(2->256) + LSTMCell) over obs_traj_rel, carry (h0, c0)
  c_out = h1.T.reshape(B, H)
  scan2: 12 steps over pre_traj_rel, carry (h1, c1)
  x_out = h2.T.reshape(B, H)

Strategy (data-parallel over batch, 8 NeuronCores, BL=512 rows each):
  - The 256-wide input embedding is folded into the gate weights on the host:
      gates = x @ (W_ih @ W_in).T + h @ W_hh.T + (W_ih @ b_in + b)
    so the per-step device work is one K=2 matmul + one K=1024 matmul.
  - Hidden state lives in SBUF transposed ([H, BL]); gates come out as
    [4H, BL] tiles. Matmul weights are stored pre-scaled by 512 in every
    representation, and every gate activation applies scale=1/512, so fp8
    and bf16 paths can mix freely in one PSUM accumulation.
  - Precision schedule (budget: rel err < 2e-2): scan-1 amplifies injected
    errors ~1.2x/step, so its step 0 runs all-bf16; every other step (1-7 of
    scan-1, all of scan-2) runs the f/i/o gate tiles in fp8e4 DoubleRow
    (two 128-row contraction blocks per matmul -> ~2x PE rate) and keeps
    the g gate in bf16: the tanh-path is ~10x more sensitive to
    quantization noise than the sigmoid gates (measured: ifo-fp8 adds
    ~5e-3 to x_out, g-fp8 alone adds ~3.8e-2; HW errors match the numpy
    simulation to 3 digits). h is kept both as bf16 tiles (g matmuls) and
    as an fp8 k-pair tile (ifo matmuls); weights W*512 fit e4m3 (max
    |W*512| ~ 131 < 240) so no clipping loss.
  - The four K=2 input-projection matmuls of a j-group are row-packed into
    disjoint 32-row strips of the PE array (tile_position=(32r,0)), so they
    run concurrently (~1 matmul's streaming time instead of 4).
  - Weight columns are permuted on the host so each j-group's gate blocks
    are contiguous; weights DMA in (k, j) chunks so compute starts after
    ~1MB instead of the full set.
  - h is written back as bf16 (next step's matmul operand); the scan-final h
    is additionally produced in fp32 and DMA'd out as [H, BL]; the host
    concatenation of those per-core blocks is exactly h.T, so c_out/x_out are
    free reshapes.
"""

import numpy as np
import ml_dtypes

T_OBS, T_PRE, B = 8, 12, 4096
FEAT, H = 2, 1024
N_CORES = 8
BL = B // N_CORES        # 512 batch rows per core
KB = H // 128            # 8 contraction blocks over H
NG = 4 * H // 128        # 32 gate row-tiles over 4H
WS = 512.0               # weight pre-scale; activations use scale=1/WS

# Gate-block permutation: new column block p = 4*j + gi holds original gate
# block (f,i,g,o)[gi] for H-rows j*128..(j+1)*128, i.e. original m-index
# _PERM[p]. Gates appear in (f,i,g,o) order so the forget gate (first in the
# cell-update chain) finishes earliest.
_GATE_ORIG = (1, 0, 2, 3)  # f,i,g,o -> original gate index (i,f,g,o order)
_PERM = [g0 * KB + j for j in range(KB) for g0 in _GATE_ORIG]
# fp8 weight tensor for scan 2: only the f,i,o blocks, 3 per j-group
_IFO_ORIG = (1, 0, 3)
_PERM8 = [g0 * KB + j for j in range(KB) for g0 in _IFO_ORIG]
# bf16 g-gate weight tensor for scan 2
_PERMG = [2 * KB + j for j in range(KB)]

_BF16 = ml_dtypes.bfloat16
_F8E4 = ml_dtypes.float8_e4m3
_CACHE = {}
# DoubleRowSwInterleave: weights pre-interleaved on the host
# (A127,B127,A126,B126,... per k-pair) so the weight load is a contiguous
# read instead of the hardware interleave pattern.
_SWI = False


def _swi_pack(w8):
    """[H, 24*128] fp8 block layout -> [4, 128, 24*256] sw-interleaved."""
    w = np.asarray(w8)
    # w[128*kb + p, q*128 + m]
    w4 = w.reshape(4, 2, 128, 24, 128)          # [b, i, p, q, m]
    rev = w4[:, :, :, :, ::-1]                  # m -> 127-u
    # target [b, p, q, u, i]
    out = rev.transpose(0, 2, 3, 4, 1)
    return np.ascontiguousarray(out.reshape(4, 128, 24 * 256))


def _build_nc():
    import concourse.tile as tile
    from concourse import bacc, mybir

    f32 = mybir.dt.float32
    bf16 = mybir.dt.bfloat16
    f8e4 = mybir.dt.float8e4
    SIG = mybir.ActivationFunctionType.Sigmoid
    TANH = mybir.ActivationFunctionType.Tanh
    DR = mybir.MatmulPerfMode.DoubleRow

    nc = bacc.Bacc(
        "TRN2", target_bir_lowering=False, debug=False, enable_asserts=False
    )

    DR_MODE = (
        mybir.MatmulPerfMode.DoubleRowSwInterleave if _SWI else DR
    )
    w8_dram_shape = [4, 128, 24 * 256] if _SWI else [H, 24 * 128]
    d_w0 = nc.dram_tensor("w_obs", [H, 4 * H], bf16, kind="ExternalInput").ap()
    d_w8o = nc.dram_tensor("w8_obs", w8_dram_shape, f8e4, kind="ExternalInput").ap()
    d_w8 = nc.dram_tensor("w8_pre", w8_dram_shape, f8e4, kind="ExternalInput").ap()
    d_wg = nc.dram_tensor("wg_pre", [H, 8 * 128], bf16, kind="ExternalInput").ap()
    d_wx = [
        nc.dram_tensor(f"wx_{s}", [128, 4 * H], bf16, kind="ExternalInput").ap()
        for s in ("obs", "pre")
    ]
    d_bias = [
        nc.dram_tensor(f"bias_{s}", [128, NG], f32, kind="ExternalInput").ap()
        for s in ("obs", "pre")
    ]
    d_x = [
        nc.dram_tensor(f"x_{s}", [t, FEAT, BL], bf16, kind="ExternalInput").ap()
        for s, t in zip(("obs", "pre"), (T_OBS, T_PRE))
    ]
    d_h0 = nc.dram_tensor("h0T", [H, BL], bf16, kind="ExternalInput").ap()
    d_c0 = nc.dram_tensor("c0T", [H, BL], f32, kind="ExternalInput").ap()
    d_c1 = nc.dram_tensor("c1T", [H, BL], f32, kind="ExternalInput").ap()
    d_hout = [
        nc.dram_tensor(f"h{i}T", [H, BL], f32, kind="ExternalOutput").ap()
        for i in (1, 2)
    ]

    with tile.TileContext(nc) as tc:
        with (
            tc.tile_pool(name="wp", bufs=1) as wp,
            tc.tile_pool(name="hp", bufs=18) as hp,
            tc.tile_pool(name="h8p", bufs=2) as h8p,
            tc.tile_pool(name="cp", bufs=1) as cp,
            tc.tile_pool(name="gp", bufs=9) as gp,
            tc.tile_pool(name="xp", bufs=3) as xp,
            tc.tile_pool(name="pp", bufs=2, space="PSUM") as pp,
        ):
            # Persistent weights. Scan-1 set loads in (j-chunk)-major order so
            # the first j-group can start after ~1MB; scan-2 set is emitted
            # inside the scan loop (after step 1) so its DMA traffic queues
            # behind the critical first-step loads.
            w0_sb = [
                wp.tile([128, 4 * H], bf16, tag=f"w0_{k}", name=f"w0_{k}")
                for k in range(KB)
            ]
            w8_shape = [128, 4, 24, 2, 128] if _SWI else [128, KB, 24 * 128]
            w8o_sb = wp.tile(w8_shape, f8e4, tag="w8o", name="w8o")
            w8_sb = wp.tile(w8_shape, f8e4, tag="w8", name="w8")

            def load_w8(sb, dram):
                if _SWI:
                    for b in range(4):
                        eng = nc.sync if b % 2 == 0 else nc.vector
                        eng.dma_start(out=sb[:, b, :, :, :], in_=dram[b, :, :])
                else:
                    for k in range(KB):
                        nc.sync.dma_start(
                            out=sb[:, k, :], in_=dram[k * 128 : (k + 1) * 128, :]
                        )

            def dr_weight_ap(sb, b, q):
                if _SWI:
                    return sb[:, b, q, :, :]
                return sb[:, 2 * b : 2 * b + 2, q * 128 : (q + 1) * 128]
            wg_sb = wp.tile([128, KB, 8 * 128], bf16, tag="wg", name="wg")
            wx_sb = [
                wp.tile([128, 4 * H], bf16, tag=f"wx{s}", name=f"wx{s}")
                for s in range(2)
            ]
            bias_sb = [
                wp.tile([128, NG], f32, tag=f"bias{s}", name=f"bias{s}")
                for s in range(2)
            ]

            def load_weight_set0():
                # j-chunk-major with the wx chunk leading each j: step 0's
                # first matmuls wait for ~1.1MB, not the full wx then w0
                nc.sync.dma_start(out=bias_sb[0], in_=d_bias[0][:, :])
                for j in range(KB):
                    jsl = slice(j * 512, (j + 1) * 512)
                    nc.sync.dma_start(out=wx_sb[0][:, jsl], in_=d_wx[0][:, jsl])
                    for k in range(KB):
                        nc.sync.dma_start(
                            out=w0_sb[k][:, jsl],
                            in_=d_w0[k * 128 : (k + 1) * 128, jsl],
                        )
                # fp8 ifo weights for scan-1 steps >= 1
                load_w8(w8o_sb, d_w8o)

            def load_weight_set1():
                nc.sync.dma_start(out=wx_sb[1], in_=d_wx[1][:, :])
                nc.sync.dma_start(out=bias_sb[1], in_=d_bias[1][:, :])
                load_w8(w8_sb, d_w8)
                for k in range(KB):
                    nc.sync.dma_start(
                        out=wg_sb[:, k, :], in_=d_wg[k * 128 : (k + 1) * 128, :]
                    )

            # PE warm-up: the array sits idle ~10us waiting for the first
            # weight/h0 DMAs, and would then start the real stream at the
            # HAM-throttled 1.2 GHz clock. Dummy matmuls on zeroed tiles
            # (never read) during that window flip the clock gate to 8/8
            # before the real stream begins.
            wu_w = xp.tile([128, 128], bf16, tag="x", name="wu_w")
            wu_rhs = xp.tile([128, BL], bf16, tag="x", name="wu_rhs")
            nc.vector.memset(wu_w, 0.0)
            nc.vector.memset(wu_rhs, 0.0)
            wu_p = pp.tile([128, 4 * BL], f32, tag="ps", name="wu_p")
            for _ in range(28):
                nc.tensor.matmul(
                    wu_p[:, :BL], wu_w, wu_rhs, start=True, stop=True
                )

            # h0/c0 first so they lead the scalar DMA queue (w0's odd-k
            # chunks share it and must not starve them)
            h_cur = [
                hp.tile([128, BL], bf16, tag="h", name=f"h_init_{k}")
                for k in range(KB)
            ]
            for k in range(KB):
                nc.scalar.dma_start(out=h_cur[k], in_=d_h0[k * 128 : (k + 1) * 128, :])
            c_sb = [
                cp.tile([128, BL], f32, tag=f"c{j}", name=f"c{j}") for j in range(KB)
            ]
            for j in range(KB):
                nc.scalar.dma_start(out=c_sb[j], in_=d_c0[j * 128 : (j + 1) * 128, :])

            load_weight_set0()

            h8_cur = None
            steps = []
            for s, T, d_xs, d_out in (
                (0, T_OBS, d_x[0], d_hout[0]),
                (1, T_PRE, d_x[1], d_hout[1]),
            ):
                for t in range(T):
                    steps.append((s, t, T, d_xs, d_out))

            def make_xt(si):
                s, t, T, d_xs, d_out = steps[si]
                xt = xp.tile([128, BL], bf16, tag="x", name=f"x_{s}_{t}")
                for r in range(4):
                    nc.gpsimd.dma_start(
                        out=xt[32 * r : 32 * r + FEAT, :], in_=d_xs[t, :, :]
                    )
                return xt

            def emit_x(si, xt, j):
                # One 4-bank PSUM tile per j-group: the four gate psums
                # allocate atomically, so the four row-packed K=2
                # input-projection matmuls become ready together and stream
                # concurrently in disjoint 32-row strips. Emitted one
                # j-group EARLY (pipelined), so the 4 weight-strip loads
                # ride the previous group's cheap-LDW g phase instead of
                # stalling this group's entry.
                s, t, T, d_xs, d_out = steps[si]
                big = pp.tile(
                    [128, 4 * BL], f32, tag="ps", name=f"ps_{s}_{t}_{j}"
                )
                ps = []
                for gi in range(4):
                    p = big[:, gi * BL : (gi + 1) * BL]
                    m = 4 * j + gi
                    msl = slice(m * 128, (m + 1) * 128)
                    rsl = slice(32 * gi, 32 * gi + FEAT)
                    nc.tensor.matmul(
                        p,
                        wx_sb[s][rsl, msl],
                        xt[rsl, :],
                        start=True,
                        stop=False,
                        tile_position=(32 * gi, 0),
                        skip_group_check=True,
                    )
                    ps.append(p)
                return ps

            xt_cur = make_xt(0)
            xt_next = None
            pending_ps = None

            for si, (s, t, T, d_xs, d_out) in enumerate(steps):
                if True:
                    if s == 1 and t == 0:
                        # fresh cell state for scan 2 (WAR on scan-1 reads)
                        for j in range(KB):
                            nc.scalar.dma_start(
                                out=c_sb[j], in_=d_c1[j * 128 : (j + 1) * 128, :]
                            )
                    if s == 0 and t == 1:
                        load_weight_set1()

                    last = t == T - 1
                    # step modes: scan-1 step 0 all-bf16 (early errors are
                    # amplified ~1.2x/step through the scan), everything else
                    # f/i/o-fp8 + g-bf16
                    fp8_step = not (s == 0 and t < 1)
                    produce_next = not (s == 1 and last)
                    produce_h8 = produce_next
                    h_next = (
                        [
                            hp.tile([128, BL], bf16, tag="h", name=f"h_{s}_{t}_{k}")
                            for k in range(KB)
                        ]
                        if produce_next
                        else None
                    )
                    h8_next = (
                        h8p.tile([128, KB, BL], f8e4, tag="h8", name=f"h8_{s}_{t}")
                        if produce_h8
                        else None
                    )

                    def emit_x_ahead(j):
                        # emit the x pack of the group after (si, j), if any
                        nonlocal pending_ps, xt_next
                        if j + 1 < KB:
                            pending_ps = emit_x(si, xt_cur, j + 1)
                        elif si + 1 < len(steps):
                            xt_next = make_xt(si + 1)
                            pending_ps = emit_x(si + 1, xt_next, 0)
                        else:
                            pending_ps = None

                    def emit_epilogue(j, ps, act_order):
                        # permuted layout: gate gi of j-group j is column
                        # block m = 4*j + gi, gi in (f,i,g,o) order
                        ms = [4 * j + gi for gi in range(4)]
                        bs = bias_sb[s]

                        def _act(p_in, m, func, nm):
                            o = gp.tile([128, BL], f32, tag="g", name=nm)
                            nc.scalar.activation(
                                out=o,
                                in_=p_in,
                                func=func,
                                bias=bs[:, m : m + 1],
                                scale=1.0 / WS,
                            )
                            return o

                        # emit gate activations in psum-close order (the
                        # scalar queue is strict FIFO)
                        acts = {}
                        for gi in act_order:
                            func = TANH if gi == 2 else SIG
                            acts[gi] = _act(
                                ps[gi], ms[gi], func, f"a{gi}_{s}_{t}_{j}"
                            )
                        sf, si, tg, so = (acts[gi] for gi in range(4))

                        t1 = gp.tile([128, BL], f32, tag="g", name=f"t1_{s}_{t}_{j}")
                        nc.vector.tensor_mul(t1, sf, c_sb[j])
                        t2 = gp.tile([128, BL], f32, tag="g", name=f"t2_{s}_{t}_{j}")
                        nc.vector.tensor_mul(t2, si, tg)
                        nc.vector.tensor_add(c_sb[j], t1, t2)
                        tc_j = gp.tile([128, BL], f32, tag="g", name=f"tc_{s}_{t}_{j}")
                        nc.scalar.activation(out=tc_j, in_=c_sb[j], func=TANH)

                        # next-step h first: it is on the critical path; the
                        # fp32 output copy and its DMA are not.
                        if produce_next:
                            nc.vector.tensor_mul(h_next[j], so, tc_j)
                        if produce_h8:
                            nc.vector.tensor_mul(h8_next[:, j, :], so, tc_j)
                        if last:
                            hf = gp.tile([128, BL], f32, tag="g", name=f"hf_{s}_{j}")
                            nc.vector.tensor_mul(hf, so, tc_j)
                            nc.sync.dma_start(
                                out=d_out[j * 128 : (j + 1) * 128, :], in_=hf
                            )

                    if not fp8_step:
                        for j in range(KB):
                            ps = pending_ps or emit_x(si, xt_cur, j)
                            # all-bf16 step: full K=1024 accumulation per gate
                            for gi in range(4):
                                if gi == 2:
                                    emit_x_ahead(j)
                                msl = slice(
                                    (4 * j + gi) * 128, (4 * j + gi + 1) * 128
                                )
                                for k in range(KB):
                                    nc.tensor.matmul(
                                        ps[gi],
                                        w0_sb[k][:, msl],
                                        h_cur[k],
                                        start=False,
                                        stop=(k == KB - 1),
                                        skip_group_check=True,
                                    )
                            emit_epilogue(j, ps, (0, 1, 2, 3))
                    else:
                        w8s = w8o_sb if s == 0 else w8_sb

                        def emit_g(ps, j, k):
                            if s == 0:
                                wg_ap = w0_sb[k][
                                    :, (4 * j + 2) * 128 : (4 * j + 3) * 128
                                ]
                            else:
                                wg_ap = wg_sb[:, k, j * 128 : (j + 1) * 128]
                            nc.tensor.matmul(
                                ps[2],
                                wg_ap,
                                h_cur[k],
                                start=False,
                                stop=(k == KB - 1),
                                skip_group_check=True,
                            )

                        # Per j-group: x pack, then the g chain (cheap FWL
                        # weight loads let the LDW port prefetch the 256-col
                        # DoubleRow weights during the g stream), then the
                        # f/i/o DoubleRow chains round-robined; the NEXT
                        # group's x pack emits between g and DR. (Tested
                        # worse: interleaving g into the DR phase — each
                        # Normal<->DoubleRow transition costs ~100ns — and
                        # pair-wise j-group phases, which serialize on PSUM.)
                        for j in range(KB):
                            ps = pending_ps or emit_x(si, xt_cur, j)
                            for k in range(KB):
                                emit_g(ps, j, k)
                            emit_x_ahead(j)
                            for b in range(KB // 2):
                                for gidx, gi in enumerate((0, 1, 3)):
                                    nc.tensor.matmul(
                                        ps[gi],
                                        dr_weight_ap(w8s, b, 3 * j + gidx),
                                        h8_cur[:, 2 * b : 2 * b + 2, :],
                                        start=False,
                                        stop=(b == KB // 2 - 1),
                                        perf_mode=DR_MODE,
                                        skip_group_check=True,
                                    )
                            emit_epilogue(j, ps, (2, 0, 1, 3))

                    if xt_next is not None:
                        xt_cur = xt_next
                        xt_next = None
                    if produce_next:
                        h_cur = h_next
                    if produce_h8:
                        h8_cur = h8_next

    nc.compile()
    return nc


def _prep_host(inputs):
    inputs = {k: np.asarray(v) for k, v in inputs.items()}
    f32 = np.float32
    W_in = inputs["W_in"].astype(np.float64)
    b_in = inputs["b_in"].astype(np.float64)

    shared = {}
    for tag in ("obs", "pre"):
        W_ih = inputs[f"W_ih_{tag}"].astype(np.float64)
        W_hh = inputs[f"W_hh_{tag}"].astype(f32)
        b = inputs[f"b_{tag}"].astype(np.float64)
        W_eff = (W_ih @ W_in).astype(f32)        # [4H, FEAT]
        b_eff = (W_ih @ b_in + b).astype(f32)    # [4H]
        # weight representations pre-scaled by WS; activations divide back
        wT = np.ascontiguousarray(W_hh.T) * f32(WS)          # [H, 4H]
        wT_blocks = wT.reshape(H, NG, 128)
        w8 = np.clip(
            np.ascontiguousarray(wT_blocks[:, _PERM8, :].reshape(H, 24 * 128)),
            -240.0,
            240.0,
        ).astype(_F8E4)
        shared[f"w8_{tag}"] = _swi_pack(w8) if _SWI else w8
        if tag == "obs":
            w_p = wT_blocks[:, _PERM, :].reshape(H, 4 * H)
            shared["w_obs"] = np.ascontiguousarray(w_p).astype(_BF16)
        else:
            wg = wT_blocks[:, _PERMG, :].reshape(H, 8 * 128)
            shared["wg_pre"] = np.ascontiguousarray(wg).astype(_BF16)
        wx_p = np.zeros((128, 4 * H), f32)
        wx_src = (W_eff.T * WS).reshape(FEAT, NG, 128)[:, _PERM, :].reshape(
            FEAT, 4 * H
        )
        for r in range(4):
            wx_p[32 * r : 32 * r + FEAT] = wx_src
        bias_p = b_eff.reshape(NG, 128)[_PERM, :].T  # [128, NG]
        shared[f"wx_{tag}"] = np.ascontiguousarray(wx_p).astype(_BF16)
        shared[f"bias_{tag}"] = np.ascontiguousarray(bias_p)

    obs = inputs["obs_traj_rel"].astype(f32)
    pre = inputs["pre_traj_rel"].astype(f32)
    h0 = inputs["h0"].astype(f32)
    c0 = inputs["c0"].astype(f32)
    c1 = inputs["c1"].astype(f32)

    in_maps = []
    for c in range(N_CORES):
        sl = slice(c * BL, (c + 1) * BL)
        m = dict(shared)
        m["x_obs"] = np.ascontiguousarray(obs[:, sl, :].transpose(0, 2, 1)).astype(
            _BF16
        )
        m["x_pre"] = np.ascontiguousarray(pre[:, sl, :].transpose(0, 2, 1)).astype(
            _BF16
        )
        m["h0T"] = np.ascontiguousarray(h0[sl].T).astype(_BF16)
        m["c0T"] = np.ascontiguousarray(c0[sl].T)
        m["c1T"] = np.ascontiguousarray(c1[sl].T)
        in_maps.append(m)
    return in_maps


def _run(inputs, trace=False):
    from concourse import bass_utils

    nc = _CACHE.get("nc")
    if nc is None:
        nc = _build_nc()
        _CACHE["nc"] = nc
    in_maps = _prep_host(inputs)
    res = bass_utils.run_bass_kernel_spmd(
        nc, in_maps, core_ids=list(range(N_CORES)), trace=trace
    )
    h1 = np.concatenate([r["h1T"] for r in res.results], axis=1)  # [H, B] == h1.T
    h2 = np.concatenate([r["h2T"] for r in res.results], axis=1)
    c_out = np.ascontiguousarray(h1.reshape(B, H), dtype=np.float32)
    x_out = np.ascontiguousarray(h2.reshape(B, H), dtype=np.float32)
    return (c_out, x_out), res


def kernel(**inputs):
    out, _ = _run(inputs, trace=False)
    return out
